# revision 34
# baseline (speedup 1.0000x reference)
"""GCGRU (Chebyshev graph-conv GRU) on 8 Trainium2 NeuronCores.

Sharding: node/tensor-parallel. Core j owns nodes [128j, 128j+128); batch is
replicated. Per core, on device:
  A      = softmax(relu(emb @ emb.T))               (full, replicated compute)
  S1     = A rows (local), S2 = (2 A_loc @ A - I) rows (local); stored
           transposed per 128-m-chunk as S_T[m, chunk, k, n] (f16)
  w_loc  = einsum('nd,dkio->nkio', emb_loc, W_pool) (local nodes, on the fly,
           kept in SBUF f16; gate generated in two 64-wide o-halves)
  x_g    = einsum('knm,bmc->bnkc', S, inp)          (local n, all b; computed
           transposed: X[c, b, k, n], contraction over m streamed from DRAM)
  z_r    = per-node matmuls over (k,i) + bias, sigmoid; rs = r * state_loc
  AllGather(rs) -> full cand right-half for the update-phase x_g
  h_hat  = tanh(update gconv), out = z*state + (1-z)*h_hat   (local nodes)
Host: slices/casts/interleaves inputs per core (DMA-friendly layouts),
concatenates per-core outputs over nodes.

DMA layouts: phase-1 streams inpr16[m%128, b, 8*(x|s)] so one 2KB/partition
DMA feeds all 8 contraction chunks of a batch. Phase-2 streams xr16 the same
way; the gathered rs half rides separate tiles and contributes via col-tiled
matmuls into psum partitions 64:127.
"""
import os
import sys

if "/opt/trn_rl_repo" not in sys.path:
    sys.path.insert(0, "/opt/trn_rl_repo")

import numpy as np

import concourse.bass as bass
import concourse.mybir as mybir
import concourse.tile as tile
from concourse import bacc
from concourse.bass_utils import run_bass_kernel_spmd
from concourse.masks import make_identity

F32 = mybir.dt.float32
F16 = mybir.dt.float16
U8 = mybir.dt.uint8

R = 8          # cores
B = 64         # batch
N = 1024       # nodes
NL = N // R    # nodes per core = 128
H = 64         # hidden (= D_in = D_out)
C = 2 * H      # gconv input channels = 128
E = 16         # embedding dim
KC = 3         # chebyshev order
MC = 8         # m-chunks of 128
OG = 2 * H     # gate output channels = 128
ACT = mybir.ActivationFunctionType
ALU = mybir.AluOpType

_CACHED_NC = None


def _softmax_rows(nc, pool, ap, width):
    """Row softmax over the free dim of an SBUF [128, width] f32 AP.

    No max subtraction: inputs are relu(emb@emb.T) <= ~60, exp fits f32."""
    ssum = pool.tile([128, 1], F32, tag="sm_sum")
    nc.scalar.activation(ap, ap, ACT.Exp, accum_out=ssum[:])
    rcp = pool.tile([128, 1], F32, tag="sm_rcp")
    nc.vector.reciprocal(rcp[:], ssum[:])
    nc.vector.tensor_scalar_mul(ap, ap, rcp[:])


def build_program(stop=""):
    stop = stop or os.environ.get("K_STOP", "")
    nc = bacc.Bacc("TRN2", target_bir_lowering=False, debug=False, num_devices=R)

    # [m % 128, b, mc*C + c]; c = [x | state]: one DMA per b covers all chunks
    inpr16 = nc.dram_tensor("inpr16", [128, B, MC * C], F16, kind="ExternalInput")
    # [m % 128, b, mc*H + h]: phase-2 x-halves
    xr16 = nc.dram_tensor("xr16", [128, B, MC * H], F16, kind="ExternalInput")
    # state/255 in the same interleave (recombines with gathered uint8 r)
    sr255 = nc.dram_tensor("sr255", [128, B, MC * H], F16, kind="ExternalInput")
    inp16_loc = nc.dram_tensor("inp16_loc", [B, NL, C], F16, kind="ExternalInput")
    x16_loc = nc.dram_tensor("x16_loc", [B, NL, H], F16, kind="ExternalInput")
    state_loc = nc.dram_tensor("state_loc", [B, NL * H], F32, kind="ExternalInput")
    embT = nc.dram_tensor("embT", [E, N], F32, kind="ExternalInput")
    embT_loc = nc.dram_tensor("embT_loc", [E, NL], F32, kind="ExternalInput")
    embT16_loc = nc.dram_tensor("embT16_loc", [E, NL], F16, kind="ExternalInput")
    wg16 = nc.dram_tensor("wg16", [KC, E, OG, C], F16, kind="ExternalInput")
    wu16 = nc.dram_tensor("wu16", [KC, E, H, C], F16, kind="ExternalInput")
    bg = nc.dram_tensor("bg", [E, OG], F32, kind="ExternalInput")
    bu = nc.dram_tensor("bu", [E, H], F32, kind="ExternalInput")
    eye_loc = nc.dram_tensor("eye_loc", [NL, N], F32, kind="ExternalInput")
    out_loc = nc.dram_tensor("out_loc", [B, NL * H], F32, kind="ExternalOutput")

    with tile.TileContext(nc) as tc:
        with (
            tc.tile_pool(name="glob", bufs=1) as glob,
            tc.tile_pool(name="dram", bufs=1, space="DRAM") as dram,
            tc.tile_pool(name="psT", bufs=2, space="PSUM") as psT,
        ):
            rs_dram = dram.tile([B, NL, H], U8)        # quantized r (x255)
            rsloc_dram = dram.tile([B, NL, H], F16)    # r*state for local cand
            ag_dram = [
                dram.tile([R, B // 4, NL, H], U8, addr_space="Shared",
                          name=f"ag_dram{i}")
                for i in range(4)
            ]
            z_dram = dram.tile([B, NL * H], F16)
            biasg_dram = dram.tile([NL, OG], F16)
            biasu_dram = dram.tile([NL, H], F16)

            ident16 = glob.tile([128, 128], F16)
            make_identity(nc, ident16[:])
            embTl_sb = glob.tile([E, NL], F32)
            nc.sync.dma_start(embTl_sb[:], embT_loc[:])
            embTl16_sb = glob.tile([E, NL], F16)
            nc.sync.dma_start(embTl16_sb[:], embT16_loc[:])
            # S_T[m, chunk, kk, n] = S_{kk+1}[n0+n, 128*chunk+m], f16
            S_T = glob.tile([128, MC, 2, NL], F16)

            # ---------------- stage A: supports --------------------------
            with (
                tc.tile_pool(name="stgA", bufs=1) as pA,
                tc.tile_pool(name="psA", bufs=2, space="PSUM") as psA,
            ):
                ident32 = pA.tile([128, 128], F32)
                make_identity(nc, ident32[:])
                embT_sb = pA.tile([E, N], F32)
                nc.sync.dma_start(embT_sb[:], embT[:])
                eye_sb = pA.tile([NL, N], F32)
                nc.sync.dma_start(eye_sb[:], eye_loc[:])

                A_sb = pA.tile([128, MC, N], F32)  # A[128 t + p, m] at [p, t, m]
                for t in range(MC):
                    for ch in range(2):
                        ps = psA.tile([128, 512], F32, name=f"psa_{t}_{ch}", tag="psa")
                        nc.tensor.matmul(
                            ps[:], embT_sb[:, t * 128:(t + 1) * 128],
                            embT_sb[:, ch * 512:(ch + 1) * 512],
                        )
                        nc.vector.tensor_scalar_max(
                            A_sb[:, t, ch * 512:(ch + 1) * 512], ps[:], 0.0)
                    _softmax_rows(nc, pA, A_sb[:, t, :], N)

                # A rows for local nodes (recomputed to stay rank-agnostic)
                Aloc_sb = pA.tile([NL, N], F32)
                for ch in range(2):
                    ps = psA.tile([128, 512], F32, name=f"psl_{ch}", tag="psa")
                    nc.tensor.matmul(
                        ps[:], embTl_sb[:], embT_sb[:, ch * 512:(ch + 1) * 512])
                    nc.vector.tensor_scalar_max(
                        Aloc_sb[:, ch * 512:(ch + 1) * 512], ps[:], 0.0)
                _softmax_rows(nc, pA, Aloc_sb[:, :], N)

                # S1_T chunks (transpose A_loc), keep f32 copy for the T2 matmul
                s1t32 = pA.tile([128, MC, 128], F32)
                for mc in range(MC):
                    tp = psA.tile([128, 128], F32, name=f"t1_{mc}", tag="tpA")
                    nc.tensor.transpose(
                        tp[:], Aloc_sb[:, mc * 128:(mc + 1) * 128], ident32[:])
                    nc.vector.tensor_copy(s1t32[:, mc, :], tp[:])
                    nc.vector.tensor_copy(S_T[:, mc, 0, :], tp[:])

                # T2 rows = 2 * A_loc @ A - I_loc
                T2sb = pA.tile([NL, N], F32)
                for ch in range(2):
                    ps = psA.tile([128, 512], F32, name=f"pst2_{ch}", tag="psa")
                    for mc in range(MC):
                        nc.tensor.matmul(
                            ps[:], s1t32[:, mc, :],
                            A_sb[:, mc, ch * 512:(ch + 1) * 512],
                            start=(mc == 0), stop=(mc == MC - 1),
                        )
                    sl = slice(ch * 512, (ch + 1) * 512)
                    nc.vector.tensor_scalar_mul(T2sb[:, sl], ps[:], 2.0)
                    nc.vector.tensor_sub(T2sb[:, sl], T2sb[:, sl], eye_sb[:, sl])
                for mc in range(MC):
                    tp = psA.tile([128, 128], F32, name=f"t2_{mc}", tag="tpA")
                    nc.tensor.transpose(
                        tp[:], T2sb[:, mc * 128:(mc + 1) * 128], ident32[:])
                    nc.vector.tensor_copy(S_T[:, mc, 1, :], tp[:])

                # biases: bias[n, o] = sum_d emb[n, d] * b_pool[d, o]
                bg_sb = pA.tile([E, OG], F32)
                nc.sync.dma_start(bg_sb[:], bg[:])
                bu_sb = pA.tile([E, H], F32)
                nc.sync.dma_start(bu_sb[:], bu[:])
                bgp = psA.tile([128, OG], F32, name="bgp", tag="tpA")
                nc.tensor.matmul(bgp[:], embTl_sb[:], bg_sb[:])
                bg16 = pA.tile([NL, OG], F16)
                nc.vector.tensor_copy(bg16[:], bgp[:])
                nc.sync.dma_start(biasg_dram[:], bg16[:])
                bup = psA.tile([128, H], F32, name="bup", tag="tpA")
                nc.tensor.matmul(bup[:], embTl_sb[:], bu_sb[:])
                bu16 = pA.tile([NL, H], F16)
                nc.vector.tensor_copy(bu16[:], bup[:])
                nc.sync.dma_start(biasu_dram[:], bu16[:])

            # ---------------- main phases --------------------------------
            X = glob.tile([128, B, KC, NL], F16)       # x_g^T: [c, b, k, n]
            W_sb = glob.tile([128, KC, H, NL], F16)    # w_loc: [i, k, o, n]
            staged = glob.tile([B, NL * H], F16)       # per-pass raw gconv out

            with (
                tc.tile_pool(name="stream", bufs=1) as pS,
                tc.tile_pool(name="psX", bufs=2, space="PSUM") as psX,
                tc.tile_pool(name="psF", bufs=2, space="PSUM") as psF,
            ):
                def transp_c0(phase, b4):
                    """X[:, b, 0, :] for 4 batches: local k=0 chunk transposed."""
                    c04 = pS.tile([128, 4, C], F16, name=f"c04_{phase}_{b4}",
                                  tag="c04", bufs=2)
                    if phase == 0:
                        nc.gpsimd.dma_start(
                            c04[:], inp16_loc[:].rearrange(
                                "b n c -> n b c")[:, b4:b4 + 4, :])
                    else:
                        nc.gpsimd.dma_start(
                            c04[:, :, :H], x16_loc[:].rearrange(
                                "b n c -> n b c")[:, b4:b4 + 4, :])
                        nc.gpsimd.dma_start(
                            c04[:, :, H:], rsloc_dram[:].rearrange(
                                "b n h -> n b h")[:, b4:b4 + 4, :])
                    tp4 = psT.tile([128, 4, 128], F16,
                                   name=f"tp4_{phase}_{b4}", tag="tp4")
                    for i in range(4):
                        nc.tensor.transpose(tp4[:, i, :], c04[:, i, :], ident16[:])
                    nc.vector.tensor_copy(
                        X[:, b4:b4 + 4, 0, :], tp4[:])

                def x_g_phase0():
                    for b4 in range(0, B, 4):
                        transp_c0(0, b4)
                    for b4 in range(0, B, 4):
                        lh4 = pS.tile([128, 4, MC, C], F16,
                                      name=f"lh_{b4}", tag="stream4b", bufs=2)
                        nc.sync.dma_start(
                            lh4[:], inpr16[:, b4:b4 + 4, :].rearrange(
                                "p b (mc c) -> p b mc c", c=C))
                        for i2 in range(0, 4, 2):
                            pxg = psX.tile([128, 2, 2, NL], F32,
                                           name=f"pxg0_{b4}_{i2}", tag="pxg")
                            for i in range(2):
                                for mc in range(MC):
                                    nc.tensor.matmul(
                                        pxg[:, i, :, :], lh4[:, i2 + i, mc, :],
                                        S_T[:, mc, :, :],
                                        start=(mc == 0), stop=(mc == MC - 1))
                            if (i2 // 2) % 2 == 0:
                                nc.vector.tensor_copy(
                                    X[:, b4 + i2:b4 + i2 + 2, 1:3, :], pxg[:])
                            else:
                                nc.scalar.copy(
                                    X[:, b4 + i2:b4 + i2 + 2, 1:3, :], pxg[:])

                def x_g_phase1():
                    for b4 in range(0, B, 4):
                        transp_c0(1, b4)
                    for bg in range(0, B, 16):
                        agb = pS.tile([128, MC, 16, H], U8,
                                      name=f"ag_{bg}", tag="agb", bufs=2)
                        bsz = B // NAG
                        ag_src = ag_dram[bg // bsz][:, bg % bsz:bg % bsz + 16]\
                            .rearrange("r b n h -> n r b h")
                        nc.gpsimd.dma_start(agb[:, :MC // 2], ag_src[:, :MC // 2])
                        nc.gpsimd.dma_start(agb[:, MC // 2:], ag_src[:, MC // 2:])
                        for b4 in range(bg, bg + 16, 4):
                            xr4 = pS.tile([128, 4, MC, H], F16,
                                          name=f"xr4_{b4}", tag="xr4", bufs=2)
                            nc.sync.dma_start(
                                xr4[:],
                                xr16[:, b4:b4 + 4, :].rearrange(
                                    "p b (mc h) -> p b mc h", h=H))
                            sr4 = pS.tile([128, 4, MC, H], F16,
                                          name=f"sr4_{b4}", tag="stream4b",
                                          bufs=2)
                            nc.scalar.dma_start(
                                sr4[:],
                                sr255[:, b4:b4 + 4, :].rearrange(
                                    "p b (mc h) -> p b mc h", h=H))
                            for i in range(4):
                                b = b4 + i
                                rsb = pS.tile([128, MC, H], F16,
                                              name=f"rsb_{b}", tag="rsb",
                                              bufs=2)
                                nc.vector.tensor_mul(
                                    rsb[:], agb[:, :, b - bg, :], sr4[:, i, :, :])
                                if i % 2 == 0:
                                    pxg = psX.tile([128, 2, 2, NL], F32,
                                                   name=f"pxg1_{b}", tag="pxg")
                                for mc in range(MC):
                                    nc.tensor.matmul(
                                        pxg[:64, i % 2, :, :],
                                        xr4[:, i, mc, :],
                                        S_T[:, mc, :, :],
                                        start=(mc == 0), stop=(mc == MC - 1),
                                        tile_position=(0, 0))
                                    nc.tensor.matmul(
                                        pxg[64:, i % 2, :, :],
                                        rsb[:, mc, :],
                                        S_T[:, mc, :, :],
                                        start=(mc == 0), stop=(mc == MC - 1),
                                        tile_position=(0, 64))
                                if i % 2 == 1:
                                    if (b // 2) % 2 == 0:
                                        nc.vector.tensor_copy(
                                            X[:, b - 1:b + 1, 1:3, :], pxg[:])
                                    else:
                                        nc.scalar.copy(
                                            X[:, b - 1:b + 1, 1:3, :], pxg[:])

                def wgen(wpool, o0, label):
                    # W_sb[i, k, oi, n] = sum_d wpool[k, d, o0+oi, i]*emb_loc[n, d]
                    for k in range(KC):
                        wk = pS.tile([E, H, C], F16, name=f"wk_{label}_{k}",
                                     tag="wk", bufs=1)
                        nc.sync.dma_start(wk[:], wpool[k, :, o0:o0 + H, :])
                        for o4 in range(0, H, 4):
                            wp4 = psT.tile([128, 4, NL], F32,
                                           name=f"wp_{label}_{k}_{o4}", tag="wp4")
                            for i in range(4):
                                nc.tensor.matmul(
                                    wp4[:, i, :], wk[:, o4 + i, :],
                                    embTl16_sb[:])
                            if (o4 // 4) % 2 == 0:
                                nc.vector.tensor_copy(
                                    W_sb[:, k, o4:o4 + 4, :], wp4[:])
                            else:
                                nc.scalar.copy(
                                    W_sb[:, k, o4:o4 + 4, :], wp4[:])

                def final_pass(label, bh=None):
                    # bh=(b0, nb): restrict to batches [b0, b0+nb) (col-tiled)
                    b0, nb = (0, B) if bh is None else bh
                    for n8 in range(0, NL, 8):
                        fp8 = psF.tile([B, 8, H], F32,
                                       name=f"fp_{label}_{n8}", tag="fp8")
                        for nn in range(8):
                            n = n8 + nn
                            for k in range(KC):
                                kw = {}
                                if bh is not None:
                                    kw["tile_position"] = (0, b0)
                                nc.tensor.matmul(
                                    fp8[b0:b0 + nb, nn, :],
                                    X[:, b0:b0 + nb, k, n], W_sb[:, k, :, n],
                                    start=(k == 0), stop=(k == KC - 1), **kw)
                        nc.vector.tensor_copy(
                            staged[b0:b0 + nb, n8 * H:(n8 + 8) * H],
                            fp8[b0:b0 + nb, :, :])

                QW = NL * H // 4  # chunk width for element-wise tails

                def bias_add_q(bias_dram_t, o0, q, label):
                    # adds bias to staged[:, q*QW:(q+1)*QW] (32 nodes per chunk)
                    bflat = pS.tile([1, QW], F16, name=f"bf_{label}_{q}",
                                    tag="bflat", bufs=1)
                    nq = QW // H
                    nc.scalar.dma_start(
                        bflat[:].rearrange("p (n h) -> p n h", h=H),
                        bias_dram_t[q * nq:(q + 1) * nq, o0:o0 + H])
                    brep = pS.tile([B, QW], F16, name=f"brp_{label}_{q}",
                                   tag="brep", bufs=1)
                    nc.gpsimd.partition_broadcast(brep[:], bflat[:])
                    sl = slice(q * QW, (q + 1) * QW)
                    nc.vector.tensor_add(staged[:, sl], staged[:, sl], brep[:])

                # ---- phase 1: gate, r-half first (o in [H, 2H)) ----
                done = [False]

                def past(mark):
                    if stop == mark:
                        done[0] = True
                    return done[0]

                if not past("stageA"):
                    x_g_phase0()
                if not past("xg0"):
                    wgen(wg16, H, "r")
                    final_pass("r")
                rq = range(0) if past("p1r") else range(4)
                for q in rq:
                    sl = slice(q * QW, (q + 1) * QW)
                    bias_add_q(biasg_dram, H, q, "r")
                    sg = pS.tile([B, QW], F32, name=f"sg_{q}", tag="ew_a", bufs=1)
                    nc.scalar.activation(sg[:], staged[:, sl], ACT.Sigmoid)
                    st = pS.tile([B, QW], F32, name=f"str_{q}", tag="ew_b", bufs=1)
                    nc.sync.dma_start(st[:], state_loc[:, sl])
                    rs = pS.tile([B, QW], F16, name=f"rs_{q}", tag="ew_d", bufs=1)
                    nc.vector.tensor_mul(rs[:], sg[:], st[:])
                    nc.scalar.dma_start(
                        rsloc_dram[:].rearrange("b n h -> b (n h)")[:, sl], rs[:])
                    r8 = pS.tile([B, QW], U8, name=f"r8_{q}", tag="ew_u8", bufs=1)
                    nc.vector.tensor_scalar_mul(r8[:], sg[:], 255.0)
                    nc.scalar.dma_start(
                        rs_dram[:].rearrange("b n h -> b (n h)")[:, sl], r8[:])

                for bq in (range(0) if done[0] else range(4)):
                    nc.gpsimd.collective_compute(
                        "AllGather", ALU.bypass,
                        replica_groups=[list(range(R))],
                        ins=[rs_dram[bq * 16:(bq + 1) * 16].opt()],
                        outs=[ag_dram[bq].opt()],
                    )

                # ---- phase 1b: gate z-half (o in [0, H)) ----
                if not past("ag"):
                    wgen(wg16, 0, "z")
                    final_pass("z")
                zq_r = range(0) if done[0] else range(4)
                for q in zq_r:
                    sl = slice(q * QW, (q + 1) * QW)
                    bias_add_q(biasg_dram, 0, q, "z")
                    zq = pS.tile([B, QW], F16, name=f"zq_{q}", tag="ew_d", bufs=1)
                    nc.scalar.activation(zq[:], staged[:, sl], ACT.Sigmoid)
                    nc.scalar.dma_start(z_dram[:, sl], zq[:])

                # ---- phase 2: update ----
                if not past("p1z"):
                    wgen(wu16, 0, "u")
                    x_g_phase1()
                if not past("xg1"):
                    final_pass("u")
                uq_r = range(0) if done[0] else range(4)
                for q in uq_r:
                    sl = slice(q * QW, (q + 1) * QW)
                    bias_add_q(biasu_dram, 0, q, "u")
                    hh = pS.tile([B, QW], F32, name=f"hh_{q}", tag="ew_a", bufs=1)
                    nc.scalar.activation(hh[:], staged[:, sl], ACT.Tanh)
                    st = pS.tile([B, QW], F32, name=f"st2_{q}", tag="ew_b", bufs=1)
                    nc.sync.dma_start(st[:], state_loc[:, sl])
                    zl = pS.tile([B, QW], F16, name=f"zl_{q}", tag="ew_d", bufs=1)
                    nc.sync.dma_start(zl[:], z_dram[:, sl])
                    # out = h + z*(state - h), with state buffer as scratch
                    nc.vector.tensor_sub(st[:], st[:], hh[:])
                    nc.vector.tensor_mul(st[:], st[:], zl[:])
                    nc.vector.tensor_add(hh[:], hh[:], st[:])
                    nc.sync.dma_start(out_loc[:, sl], hh[:])

    nc.compile()
    return nc


def _get_nc():
    global _CACHED_NC
    if _CACHED_NC is None:
        _CACHED_NC = build_program()
    return _CACHED_NC


def make_in_maps(x, state, node_embeddings, W_gate, b_gate, W_update, b_update):
    x = np.asarray(x, np.float32)
    state = np.asarray(state, np.float32)
    emb = np.asarray(node_embeddings, np.float32)
    Wg = np.asarray(W_gate, np.float32)
    Wu = np.asarray(W_update, np.float32)
    # [m%128, b, mc*C + c] interleave of concat(x, state)
    inp = np.concatenate([x, state], axis=-1)            # [B, N, C]
    inpr16 = np.ascontiguousarray(
        inp.reshape(B, MC, 128, C).transpose(2, 0, 1, 3).reshape(128, B, MC * C)
    ).astype(np.float16)
    xr16 = np.ascontiguousarray(
        x.reshape(B, MC, 128, H).transpose(2, 0, 1, 3).reshape(128, B, MC * H)
    ).astype(np.float16)
    sr255 = np.ascontiguousarray(
        (state / 255.0).reshape(B, MC, 128, H).transpose(2, 0, 1, 3)
        .reshape(128, B, MC * H)).astype(np.float16)
    # [k, d, o, i]
    wg16 = np.ascontiguousarray(Wg.transpose(1, 0, 3, 2)).astype(np.float16)
    wu16 = np.ascontiguousarray(Wu.transpose(1, 0, 3, 2)).astype(np.float16)
    embT = np.ascontiguousarray(emb.T)
    bgA = np.asarray(b_gate, np.float32)
    buA = np.asarray(b_update, np.float32)
    eyeN = np.eye(N, dtype=np.float32)
    in_maps = []
    for j in range(R):
        n0 = j * NL
        nsl = slice(n0, n0 + NL)
        in_maps.append({
            "inpr16": inpr16,
            "xr16": xr16,
            "sr255": sr255,
            "inp16_loc": np.ascontiguousarray(inp[:, nsl, :]).astype(np.float16),
            "x16_loc": np.ascontiguousarray(x[:, nsl, :]).astype(np.float16),
            "state_loc": np.ascontiguousarray(
                state[:, nsl, :]).reshape(B, NL * H),
            "embT": embT,
            "embT_loc": np.ascontiguousarray(embT[:, nsl]),
            "embT16_loc": np.ascontiguousarray(embT[:, nsl]).astype(np.float16),
            "wg16": wg16,
            "wu16": wu16,
            "bg": bgA,
            "bu": buA,
            "eye_loc": np.ascontiguousarray(eyeN[nsl, :]),
        })
    return in_maps


def kernel(x, state, node_embeddings, W_gate, b_gate, W_update, b_update):
    nc = _get_nc()
    in_maps = make_in_maps(x, state, node_embeddings, W_gate, b_gate,
                           W_update, b_update)
    res = run_bass_kernel_spmd(nc, in_maps, core_ids=list(range(R)))
    out = np.concatenate(
        [res.results[j]["out_loc"].reshape(B, NL, H) for j in range(R)], axis=1)
    return out.astype(np.float32)
